# revision 19
# baseline (speedup 1.0000x reference)
"""Self-contained Trainium2 Bass kernel for nn_MultiHeadAttention_65060164600355.

Full inputs in, full output out. Sharding: 8 cores = (batch b, query-row half),
core c -> b = c//2, query rows [1024*(c%2), 1024*(c%2)+1024). K/V projections
are split between the two cores of a batch (each computes 4 of the 8 head
pairs from its half of Wk/Wv supplied as input) and exchanged with a 2-core
HBM AllGather; rank-major gather order == global head-pair order, so the
program stays SPMD-uniform. Output assembly is pure concatenation.

Pipeline: all-bf16/fp8 matmul path (inputs cast host-side; q^T/k^T in fp8e4),
DMA-xbar transposes for X^T, SBUF-resident K^T / V / Q^T, Act engine runs exp
only, V projected directly in token-major orientation, cross-block
software-pipelined attention with the output projection injected as fillers.
"""

import numpy as np
import ml_dtypes

# ---------------------------------------------------------------------------
# Workarounds for this container's walrus build (max ONE sem-wait command per
# instruction; TileContext's end-of-kernel Drain must carry none).
# ---------------------------------------------------------------------------
import concourse.tile as tile_mod
from concourse.vector_clock import ScopedClock, VectorClock


def _drain_and_barrier(self, tick_clock, wait_clock):
    nc = self.nc
    vc = tick_clock.global_clock
    n = len(vc)
    for i in range(n):
        t = vc[i]
        if t > 0:
            vec = [0] * n
            vec[i] = t
            nop_inst = nc.sync.nop(nofuse=True, hint=f"tile_drain_wait_{i}")
            wait_clock.add_sem_waits(
                nop_inst.ins, ScopedClock({None: VectorClock(vec)})
            )
    nc.sync.drain()
    nc.all_engine_barrier()
    assert self.sems is not None
    popped = nc._tile_sem_poison_stack.pop()
    assert popped is self._sem_poison
    nc.clear_and_free_semaphores(list(self.sems.allocated().values()))
    nc.all_engine_barrier()

tile_mod.TileContext._drain_and_barrier = _drain_and_barrier

import concourse.mybir as _mybir

def legalize_waits(nc, max_waits=1):
    """This container's walrus accepts at most one sem-wait command per
    instruction. Hoist excess waits onto NoOps inserted just before the
    instruction in its basic block (same engine => same program order)."""
    ctr = 0
    for f in nc.m.functions:
        for bb in f.blocks:
            out = []
            changed = False
            for inst in bb.instructions:
                si = inst.sync_info
                if si is not None and si.on_wait and len(si.on_wait) > max_waits:
                    waits = list(si.on_wait)
                    for w in waits[:-max_waits]:
                        nop = _mybir.InstNoOp(name=f"waitfix_nop_{ctr}", ins=[], outs=[])
                        ctr += 1
                        nop.engine = inst.engine
                        nop.sync_info = _mybir.SyncInfo(on_wait=[w], on_update=[])
                        out.append(nop)
                    inst.sync_info = _mybir.SyncInfo(
                        on_wait=waits[-max_waits:], on_update=list(si.on_update)
                    )
                    changed = True
                out.append(inst)
            if changed:
                bb.instructions = out
    return ctr


# ---------------------------------------------------------------------------
# Kernel builder
# ---------------------------------------------------------------------------

from collections import deque
from contextlib import ExitStack

import concourse.bass as bass
import concourse.mybir as mybir
import concourse.tile as tile

F32 = mybir.dt.float32
BF16 = mybir.dt.bfloat16
F8 = mybir.dt.float8e4
EXP = mybir.ActivationFunctionType.Exp

REPLICA_GROUPS = [[0, 1], [2, 3], [4, 5], [6, 7]]


def build(S=2048, SQ=1024, D=1024, H=16):
    DH = 64
    assert D % 512 == 0 and S % 512 == 0 and SQ % 512 == 0 and H * DH == D
    DT = D // 128          # din tiles
    NPAIR = H // 2         # head pairs; pair i covers dout cols i*128..i*128+127
    NLOC = NPAIR // 2      # pairs computed locally (K/V split across 2 cores)
    KT = S // 128          # k tiles of 128
    QT = SQ // 128         # q tiles of 128 (output projection)
    scale = 1.0 / float(D) ** 0.5

    nc = bass.Bass()
    q_d = nc.dram_tensor("q", [SQ, D], BF16, kind="ExternalInput")
    k_d = nc.dram_tensor("k", [S, D], BF16, kind="ExternalInput")
    v_d = nc.dram_tensor("v", [S, D], BF16, kind="ExternalInput")
    # wk/wv: this core's half of the output columns (4 head pairs)
    wk_d = nc.dram_tensor("wk", [D, D // 2], BF16, kind="ExternalInput")
    wv_d = nc.dram_tensor("wv", [D, D // 2], BF16, kind="ExternalInput")
    wq_d = nc.dram_tensor("wq", [D, D], BF16, kind="ExternalInput")
    wo_d = nc.dram_tensor("wo", [D, D], BF16, kind="ExternalInput")
    out_d = nc.dram_tensor("out", [SQ, D], F32, kind="ExternalOutput")
    dn_dram = nc.dram_tensor("dn_bounce", [8, 2, 2, 512], F32)
    # K/V exchange buffers
    k_loc_d = nc.dram_tensor("k_loc", [128, NLOC, S], F8)
    v_loc_d = nc.dram_tensor("v_loc", [128, NLOC, KT, 130], BF16)
    k_all_d = nc.dram_tensor("k_all", [2, 128, NLOC, S], F8)
    v_all_d = nc.dram_tensor("v_all", [2, 128, NLOC, KT, 130], BF16)

    with tile.TileContext(nc) as tc, ExitStack() as ctx:
        # resident tensors (attention sources)
        qt_pool = ctx.enter_context(tc.tile_pool(name="qt", bufs=1))
        qt = qt_pool.tile([128, NPAIR, SQ], F8)
        kt_pool = ctx.enter_context(tc.tile_pool(name="kt", bufs=1))
        kT = kt_pool.tile([128, NPAIR, S], F8)
        vr_pool = ctx.enter_context(tc.tile_pool(name="vr", bufs=1))
        vres = vr_pool.tile([128, NPAIR, KT, 130], BF16)
        ct_pool = ctx.enter_context(tc.tile_pool(name="ct", bufs=1))
        ctxT = ct_pool.tile([128, NPAIR, SQ], BF16)

        # PSUM pools: 2 + 2*2 + 2 = 8 banks (bufs count is per tag)
        psS = ctx.enter_context(tc.tile_pool(name="psS", bufs=2, space="PSUM"))
        psC = ctx.enter_context(tc.tile_pool(name="psC", bufs=2, space="PSUM"))
        psM = ctx.enter_context(tc.tile_pool(name="psM", bufs=2, space="PSUM"))

        e_pool = ctx.enter_context(tc.tile_pool(name="e", bufs=8))
        dn_pool = ctx.enter_context(tc.tile_pool(name="dn", bufs=2))
        rb_pool = ctx.enter_context(tc.tile_pool(name="rb", bufs=1))
        out_pool = ctx.enter_context(tc.tile_pool(name="outp", bufs=2))

        def load_xt(x_dram, xt, ntok):
            # xt[:, dt, t] = x[t, dt*128+p]  (DMA xbar transpose; sync HWDGE
            # only — the Activation HWDGE queue corrupts transposes here)
            for dt in range(DT):
                nc.sync.dma_start_transpose(
                    xt[:, dt, 0:ntok],
                    x_dram[0:ntok, dt * 128:(dt + 1) * 128])

        def load_w(w_dram, pool, name, ncols):
            w = pool.tile([128, DT, ncols], BF16, tag=name, name=name)
            nc.gpsimd.dma_start(w[:], w_dram.rearrange("(t p) o -> p t o", p=128))
            return w

        def normalize(i, c, j, pcsj):
            # rows 0:64 of pcsj = unnormalized ctx^T, row 64 = denominator
            rcp = dn_pool.tile([1, 512], F32, tag="rcp", name="rcp")
            nc.vector.reciprocal(rcp[:], pcsj[64:65, :])
            dsl = dn_dram[i, c, j, :]
            nc.sync.dma_start(dsl, rcp[:])
            rb = rb_pool.tile([64, 512], F32, tag="rb", name="rb")
            bcast = bass.AP(tensor=dsl.tensor, offset=dsl.offset,
                            ap=[[0, 64]] + list(dsl.ap))
            nc.sync.dma_start(rb[:], bcast)
            nc.vector.tensor_tensor(
                ctxT[j * 64:(j + 1) * 64, i, c * 512:(c + 1) * 512],
                pcsj[:64], rb[:], mybir.AluOpType.mult)

        def drain_one(pend):
            i, c, t, j, e, pcs = pend.popleft()
            nc.tensor.matmul(
                pcs[j][:65], vres[:, i, t, j * 65:(j + 1) * 65],
                e[:], start=(t == 0), stop=(t == KT - 1))
            if t == KT - 1:
                normalize(i, c, j, pcs[j])

        def stream(blocks, fillers, lag=4):
            # continuous scores -> exp -> ctx pipeline across blocks; PE
            # stall slots are backfilled with filler thunks
            pend = deque()
            for (i, c) in blocks:
                pcs = [psC.tile([128, 512], F32, tag=f"ctx{j}", name=f"pcs{j}")
                       for j in range(2)]
                for t in range(KT):
                    for j in range(2):
                        ps = psS.tile([128, 512], F32, tag="sc", name="sc")
                        nc.tensor.matmul(
                            ps[:],
                            kT[j * 64:(j + 1) * 64, i, t * 128:(t + 1) * 128],
                            qt[j * 64:(j + 1) * 64, i, c * 512:(c + 1) * 512],
                            start=True, stop=True, tile_position=(j * 64, 0))
                        e = e_pool.tile([128, 512], BF16, tag="e", name="e")
                        nc.scalar.activation(e[:], ps[:], EXP, scale=scale)
                        pend.append((i, c, t, j, e, pcs))
                        if fillers:
                            fillers.popleft()()
                        if len(pend) > lag:
                            drain_one(pend)
            while pend:
                drain_one(pend)

        # ---- local K/V projections (4 pairs each) + exchange ----
        with ExitStack() as sKV:
            klp = sKV.enter_context(tc.tile_pool(name="kl", bufs=1))
            vlp = sKV.enter_context(tc.tile_pool(name="vl", bufs=1))
            kT_loc = klp.tile([128, NLOC, S], F8)
            vres_loc = vlp.tile([128, NLOC, KT, 130], BF16)
            nc.vector.memset(vres_loc[:, :, :, 64:65], 1.0)
            nc.vector.memset(vres_loc[:, :, :, 129:130], 1.0)

            with ExitStack() as sK:
                wK = sK.enter_context(tc.tile_pool(name="wK", bufs=1))
                xtK = sK.enter_context(tc.tile_pool(name="xtK", bufs=1))
                xtk = xtK.tile([128, DT, S], BF16, tag="xtk")
                load_xt(k_d, xtk, S)
                wk16 = load_w(wk_d, wK, "wk", D // 2)
                for lp in range(NLOC):
                    for c in range(S // 512):
                        ps = psM.tile([128, 512], F32, tag="mm", name="kps")
                        for dt in range(DT):
                            nc.tensor.matmul(
                                ps[:], wk16[:, dt, lp * 128:(lp + 1) * 128],
                                xtk[:, dt, c * 512:(c + 1) * 512],
                                start=(dt == 0), stop=(dt == DT - 1))
                        nc.vector.tensor_copy(
                            kT_loc[:, lp, c * 512:(c + 1) * 512], ps[:])

            with ExitStack() as sV:
                wV = sV.enter_context(tc.tile_pool(name="wV", bufs=1))
                xtV = sV.enter_context(tc.tile_pool(name="xtV", bufs=1))
                xtv = xtV.tile([128, DT, S], BF16, tag="xtv")
                load_xt(v_d, xtv, S)
                wv16 = load_w(wv_d, wV, "wv", D // 2)
                for tt in range(KT):
                    ps = psM.tile([128, 512], F32, tag="mm", name="vps")
                    for dt in range(DT):
                        nc.tensor.matmul(
                            ps[:], xtv[:, dt, tt * 128:(tt + 1) * 128],
                            wv16[:, dt, 0:512],
                            start=(dt == 0), stop=(dt == DT - 1))
                    for lp in range(NLOC):
                        nc.vector.tensor_copy(
                            vres_loc[:, lp, tt, 0:64],
                            ps[:, lp * 128:lp * 128 + 64])
                        nc.vector.tensor_copy(
                            vres_loc[:, lp, tt, 65:129],
                            ps[:, lp * 128 + 64:(lp + 1) * 128])

            nc.sync.dma_start(k_loc_d[:], kT_loc[:])
            nc.sync.dma_start(v_loc_d[:], vres_loc[:])
            nc.gpsimd.collective_compute(
                "AllGather", mybir.AluOpType.bypass, REPLICA_GROUPS,
                ins=[k_loc_d[:].opt()], outs=[k_all_d[:].opt()])
            nc.gpsimd.collective_compute(
                "AllGather", mybir.AluOpType.bypass, REPLICA_GROUPS,
                ins=[v_loc_d[:].opt()], outs=[v_all_d[:].opt()])

            # Q projection overlaps the collective
            with ExitStack() as sQ:
                wQ = sQ.enter_context(tc.tile_pool(name="wQ", bufs=1))
                xtQ = sQ.enter_context(tc.tile_pool(name="xtQ", bufs=1))
                xtq = xtQ.tile([128, DT, SQ], BF16, tag="xtq")
                load_xt(q_d, xtq, SQ)
                wq16 = load_w(wq_d, wQ, "wq", D)
                for p in range(NPAIR):
                    for c in range(SQ // 512):
                        ps = psM.tile([128, 512], F32, tag="mm", name="qps")
                        for dt in range(DT):
                            nc.tensor.matmul(
                                ps[:], wq16[:, dt, p * 128:(p + 1) * 128],
                                xtq[:, dt, c * 512:(c + 1) * 512],
                                start=(dt == 0), stop=(dt == DT - 1))
                        nc.vector.tensor_copy(
                            qt[:, p, c * 512:(c + 1) * 512], ps[:])

            # read the gathered halves back; rank-major == global pair order
            for sec in range(2):
                nc.sync.dma_start(
                    kT[:, sec * NLOC:(sec + 1) * NLOC, :], k_all_d[sec])
                nc.sync.dma_start(
                    vres[:, sec * NLOC:(sec + 1) * NLOC, :, :], v_all_d[sec])

        # ---- attention sweeps ----
        stream([(i, 0) for i in range(NPAIR)], deque())

        wO = ctx.enter_context(tc.tile_pool(name="wO", bufs=1))
        wo16 = load_w(wo_d, wO, "wo", D)

        def dproj_fillers(qtile):
            thunks = []
            for dc in range(D // 512):
                st = {}
                def half_a(qtile=qtile, dc=dc, st=st):
                    st["ps"] = psM.tile([128, 512], F32, tag="mm", name="ops")
                    for p in range(4):
                        nc.tensor.matmul(
                            st["ps"][:], ctxT[:, p, qtile * 128:(qtile + 1) * 128],
                            wo16[:, p, dc * 512:(dc + 1) * 512],
                            start=(p == 0), stop=False)
                def half_b(qtile=qtile, dc=dc, st=st):
                    for p in range(4, NPAIR):
                        nc.tensor.matmul(
                            st["ps"][:], ctxT[:, p, qtile * 128:(qtile + 1) * 128],
                            wo16[:, p, dc * 512:(dc + 1) * 512],
                            start=False, stop=(p == NPAIR - 1))
                    ob = out_pool.tile([128, 512], F32, tag="ob", name="ob")
                    nc.vector.tensor_copy(ob[:], st["ps"][:])
                    nc.sync.dma_start(
                        out_d[qtile * 128:(qtile + 1) * 128,
                              dc * 512:(dc + 1) * 512], ob[:])
                thunks += [half_a, half_b]
            return thunks

        fillers = deque()
        for qtile in range(4):
            fillers.extend(dproj_fillers(qtile))
        stream([(i, 1) for i in range(NPAIR)], fillers)
        assert not fillers
        for qtile in range(4, QT):
            for th in dproj_fillers(qtile):
                th()

    return nc


# ---------------------------------------------------------------------------
# Host wrapper
# ---------------------------------------------------------------------------
from concourse.bass_utils import run_bass_kernel_spmd

B, S, D, H = 4, 2048, 1024, 16
SQ = S // 2
_NC = None
PROFILE = False
TRACE_DIR = None
LAST_EXEC_NS = None


def _get_nc():
    global _NC
    if _NC is None:
        _NC = build(S=S, SQ=SQ, D=D, H=H)
        legalize_waits(_NC)
    return _NC


def kernel(queries, keys, values, Wq, Wk, Wv, Wo):
    global LAST_EXEC_NS
    nc = _get_nc()
    bf16 = ml_dtypes.bfloat16
    q16 = np.asarray(queries, dtype=bf16)
    k16 = np.asarray(keys, dtype=bf16)
    v16 = np.asarray(values, dtype=bf16)
    wq16 = np.ascontiguousarray(np.asarray(Wq, dtype=bf16))
    wk16 = np.asarray(Wk, dtype=bf16)
    wv16 = np.asarray(Wv, dtype=bf16)
    wo16 = np.ascontiguousarray(np.asarray(Wo, dtype=bf16))
    in_maps = []
    for c in range(8):
        b, half = c // 2, c % 2
        in_maps.append({
            "q": np.ascontiguousarray(q16[b, half * SQ:(half + 1) * SQ, :]),
            "k": np.ascontiguousarray(k16[b]),
            "v": np.ascontiguousarray(v16[b]),
            "wq": wq16,
            "wo": wo16,
            # this core's half of the K/V projection columns (4 head pairs)
            "wk": np.ascontiguousarray(wk16[:, half * 512:(half + 1) * 512]),
            "wv": np.ascontiguousarray(wv16[:, half * 512:(half + 1) * 512]),
        })
    res = run_bass_kernel_spmd(nc, in_maps, list(range(8)), trace=PROFILE,
                               tmpdir=TRACE_DIR)
    LAST_EXEC_NS = res.exec_time_ns
    out = np.empty((B, S, D), np.float32)
    for c in range(8):
        out[c // 2, (c % 2) * SQ:(c % 2 + 1) * SQ, :] = res.results[c]["out"]
    return out


# revision 20
# speedup vs baseline: 1.0830x; 1.0830x over previous
"""Self-contained Trainium2 Bass kernel for nn_MultiHeadAttention_65060164600355.

Full inputs in, full output out. Sharding: 8 cores = (batch b, query-row half),
core c -> b = c//2, query rows [1024*(c%2), 1024*(c%2)+1024). Each core
duplicates the K/V projections for its batch (no cross-core communication;
output assembly is pure concatenation).

v2: all-bf16 matmul path (inputs cast host-side), DMA-xbar transposes for
X^T, SBUF-resident K^T / V / Q^T (no DRAM bounce), Act engine runs exp only,
V projected directly in token-major orientation, software-pipelined
attention inner loop, V/out projections interleaved into the attention
sweeps.
"""

import numpy as np
import ml_dtypes

# ---------------------------------------------------------------------------
# Workarounds for this container's walrus build (max ONE sem-wait command per
# instruction; TileContext's end-of-kernel Drain must carry none).
# ---------------------------------------------------------------------------
import concourse.tile as tile_mod
from concourse.vector_clock import ScopedClock, VectorClock


def _drain_and_barrier(self, tick_clock, wait_clock):
    nc = self.nc
    vc = tick_clock.global_clock
    n = len(vc)
    for i in range(n):
        t = vc[i]
        if t > 0:
            vec = [0] * n
            vec[i] = t
            nop_inst = nc.sync.nop(nofuse=True, hint=f"tile_drain_wait_{i}")
            wait_clock.add_sem_waits(
                nop_inst.ins, ScopedClock({None: VectorClock(vec)})
            )
    nc.sync.drain()
    nc.all_engine_barrier()
    assert self.sems is not None
    popped = nc._tile_sem_poison_stack.pop()
    assert popped is self._sem_poison
    nc.clear_and_free_semaphores(list(self.sems.allocated().values()))
    nc.all_engine_barrier()

tile_mod.TileContext._drain_and_barrier = _drain_and_barrier

import concourse.mybir as _mybir

def legalize_waits(nc, max_waits=1):
    """This container's walrus accepts at most one sem-wait command per
    instruction. Hoist excess waits onto NoOps inserted just before the
    instruction in its basic block (same engine => same program order)."""
    ctr = 0
    for f in nc.m.functions:
        for bb in f.blocks:
            out = []
            changed = False
            for inst in bb.instructions:
                si = inst.sync_info
                if si is not None and si.on_wait and len(si.on_wait) > max_waits:
                    waits = list(si.on_wait)
                    for w in waits[:-max_waits]:
                        nop = _mybir.InstNoOp(name=f"waitfix_nop_{ctr}", ins=[], outs=[])
                        ctr += 1
                        nop.engine = inst.engine
                        nop.sync_info = _mybir.SyncInfo(on_wait=[w], on_update=[])
                        out.append(nop)
                    inst.sync_info = _mybir.SyncInfo(
                        on_wait=waits[-max_waits:], on_update=list(si.on_update)
                    )
                    changed = True
                out.append(inst)
            if changed:
                bb.instructions = out
    return ctr


# ---------------------------------------------------------------------------
# Kernel builder
# ---------------------------------------------------------------------------

from collections import deque
from contextlib import ExitStack

import concourse.bass as bass
import concourse.mybir as mybir
import concourse.tile as tile

F32 = mybir.dt.float32
F32R = mybir.dt.float32r
BF16 = mybir.dt.bfloat16
F8 = mybir.dt.float8e4
EXP = mybir.ActivationFunctionType.Exp


def build(S=2048, SQ=1024, D=1024, H=16):
    DH = 64
    assert D % 512 == 0 and S % 512 == 0 and SQ % 512 == 0 and H * DH == D
    DT = D // 128          # din tiles
    NPAIR = H // 2         # head pairs; pair i covers dout cols i*128..i*128+127
    KT = S // 128          # k tiles of 128
    QC = SQ // 512         # q chunks of 512
    QT = SQ // 128         # q tiles of 128 (phase D)
    scale = 1.0 / float(D) ** 0.5

    nc = bass.Bass()
    q_d = nc.dram_tensor("q", [SQ, D], BF16, kind="ExternalInput")
    k_d = nc.dram_tensor("k", [S, D], BF16, kind="ExternalInput")
    v_d = nc.dram_tensor("v", [S, D], BF16, kind="ExternalInput")
    w_d = {n: nc.dram_tensor(n, [D, D], BF16, kind="ExternalInput")
           for n in ("wq", "wk", "wv", "wo")}
    out_d = nc.dram_tensor("out", [SQ, D], F32, kind="ExternalOutput")
    dn_dram = nc.dram_tensor("dn_bounce", [8, 2, 2, 512], F32)

    with tile.TileContext(nc) as tc, ExitStack() as ctx:
        # resident tensors
        qt_pool = ctx.enter_context(tc.tile_pool(name="qt", bufs=1))
        qt = qt_pool.tile([128, NPAIR, SQ], F8)
        kt_pool = ctx.enter_context(tc.tile_pool(name="kt", bufs=1))
        kT = kt_pool.tile([128, NPAIR, S], F8)
        vr_pool = ctx.enter_context(tc.tile_pool(name="vr", bufs=1))
        vres = vr_pool.tile([128, NPAIR, KT, 130], BF16)
        ct_pool = ctx.enter_context(tc.tile_pool(name="ct", bufs=1))
        ctxT = ct_pool.tile([128, NPAIR, SQ], BF16)

        # PSUM pools: 2 + 2*2 + 2 = 8 banks (bufs count is per tag)
        psS = ctx.enter_context(tc.tile_pool(name="psS", bufs=2, space="PSUM"))
        psC = ctx.enter_context(tc.tile_pool(name="psC", bufs=2, space="PSUM"))
        psM = ctx.enter_context(tc.tile_pool(name="psM", bufs=2, space="PSUM"))

        e_pool = ctx.enter_context(tc.tile_pool(name="e", bufs=8))
        dn_pool = ctx.enter_context(tc.tile_pool(name="dn", bufs=2))
        rb_pool = ctx.enter_context(tc.tile_pool(name="rb", bufs=1))
        out_pool = ctx.enter_context(tc.tile_pool(name="outp", bufs=2))

        # ones columns of V (denominator rows of the ctx matmul)
        nc.vector.memset(vres[:, :, :, 64:65], 1.0)
        nc.vector.memset(vres[:, :, :, 129:130], 1.0)

        def load_xt(x_dram, xt, ntok):
            # xt[:, dt, t] = x[t, dt*128+p]  (DMA xbar transpose, one call
            # per 128-wide column block; issued on the sync HWDGE queue —
            # the Activation HWDGE queue corrupts transposes on this runtime)
            for dt in range(DT):
                nc.sync.dma_start_transpose(
                    xt[:, dt, 0:ntok],
                    x_dram[0:ntok, dt * 128:(dt + 1) * 128])

        def load_w(name, pool):
            w = pool.tile([128, DT, D], BF16, tag=name)
            nc.gpsimd.dma_start(w[:], w_d[name].rearrange("(t p) o -> p t o", p=128))
            return w

        def proj(w, xt, dst, ntok):
            # dst[:, p, tok] = (x @ W)^T restricted to pair p's 128 dout cols
            for p in range(NPAIR):
                for c in range(ntok // 512):
                    ps = psM.tile([128, 512], F32, tag="mm")
                    for dt in range(DT):
                        nc.tensor.matmul(
                            ps[:], w[:, dt, p * 128:(p + 1) * 128],
                            xt[:, dt, c * 512:(c + 1) * 512],
                            start=(dt == 0), stop=(dt == DT - 1))
                    nc.vector.tensor_copy(dst[:, p, c * 512:(c + 1) * 512], ps[:])

        def normalize(i, c, j, pcsj):
            # rows 0:64 of pcsj = unnormalized ctx^T, row 64 = denominator
            rcp = dn_pool.tile([1, 512], F32, tag="rcp", name="rcp")
            nc.vector.reciprocal(rcp[:], pcsj[64:65, :])
            dsl = dn_dram[i, c, j, :]
            nc.sync.dma_start(dsl, rcp[:])
            rb = rb_pool.tile([64, 512], F32, tag="rb", name="rb")
            bcast = bass.AP(tensor=dsl.tensor, offset=dsl.offset,
                            ap=[[0, 64]] + list(dsl.ap))
            nc.sync.dma_start(rb[:], bcast)
            nc.vector.tensor_tensor(
                ctxT[j * 64:(j + 1) * 64, i, c * 512:(c + 1) * 512],
                pcsj[:64], rb[:], mybir.AluOpType.mult)

        def drain_one(pend):
            i, c, t, j, e, pcs = pend.popleft()
            nc.tensor.matmul(
                pcs[j][:65], vres[:, i, t, j * 65:(j + 1) * 65],
                e[:], start=(t == 0), stop=(t == KT - 1))
            if t == KT - 1:
                normalize(i, c, j, pcs[j])

        def stream(blocks, fillers, lag=4):
            # continuous scores -> exp -> ctx pipeline across blocks; PE
            # stall slots are backfilled with filler thunks (K/V/out proj)
            pend = deque()
            for (i, c) in blocks:
                pcs = [psC.tile([128, 512], F32, tag=f"ctx{j}", name=f"pcs{j}")
                       for j in range(2)]
                for t in range(KT):
                    for j in range(2):
                        ps = psS.tile([128, 512], F32, tag="sc", name="sc")
                        nc.tensor.matmul(
                            ps[:],
                            kT[j * 64:(j + 1) * 64, i, t * 128:(t + 1) * 128],
                            qt[j * 64:(j + 1) * 64, i, c * 512:(c + 1) * 512],
                            start=True, stop=True, tile_position=(j * 64, 0))
                        e = e_pool.tile([128, 512], BF16, tag="e", name="e")
                        nc.scalar.activation(e[:], ps[:], EXP, scale=scale)
                        pend.append((i, c, t, j, e, pcs))
                        if fillers:
                            fillers.popleft()()
                        if len(pend) > lag:
                            drain_one(pend)
            while pend:
                drain_one(pend)

        # ---- loads ----
        # weights via gpsimd SWDGE; x^T loads spread over the two HWDGE
        # queues (sync, scalar) so no single issue queue serializes startup.
        wC = ctx.enter_context(tc.tile_pool(name="wC", bufs=1))
        xtV = ctx.enter_context(tc.tile_pool(name="xtV", bufs=1))
        xtv = xtV.tile([128, DT, S], BF16, tag="xtv")
        wv16 = load_w("wv", wC)

        with ExitStack() as sA:
            wP = sA.enter_context(tc.tile_pool(name="wP", bufs=1))
            xtQ = sA.enter_context(tc.tile_pool(name="xtQ", bufs=1))
            xtq = xtQ.tile([128, DT, SQ], BF16, tag="xtq")
            wq16 = load_w("wq", wP)
            load_xt(q_d, xtq, SQ)                                   # sync
            proj(wq16, xtq, qt, SQ)

        with ExitStack() as sB:
            wK = sB.enter_context(tc.tile_pool(name="wK", bufs=1))
            xtK = sB.enter_context(tc.tile_pool(name="xtK", bufs=1))
            xtk = xtK.tile([128, DT, S], BF16, tag="xtk")
            wk16 = load_w("wk", wK)
            load_xt(k_d, xtk, S)
            load_xt(v_d, xtv, S)

            def kproj_fillers(p):
                # K projection for pair p as 8 half-chunk thunks
                thunks = []
                for c in range(S // 512):
                    st = {}
                    def half_a(p=p, c=c, st=st):
                        st["ps"] = psM.tile([128, 512], F32, tag="mm",
                                            name="kps")
                        for dt in range(4):
                            nc.tensor.matmul(
                                st["ps"][:], wk16[:, dt, p * 128:(p + 1) * 128],
                                xtk[:, dt, c * 512:(c + 1) * 512],
                                start=(dt == 0), stop=False)
                    def half_b(p=p, c=c, st=st):
                        for dt in range(4, DT):
                            nc.tensor.matmul(
                                st["ps"][:], wk16[:, dt, p * 128:(p + 1) * 128],
                                xtk[:, dt, c * 512:(c + 1) * 512],
                                start=False, stop=(dt == DT - 1))
                        nc.vector.tensor_copy(
                            kT[:, p, c * 512:(c + 1) * 512], st["ps"][:])
                    thunks += [half_a, half_b]
                return thunks

            def vproj_tt(g, tt):
                # V in token-major orientation for pairs 4g..4g+3, k tile tt
                ps = psM.tile([128, 512], F32, tag="mm", name="vps")
                for dt in range(DT):
                    nc.tensor.matmul(
                        ps[:], xtv[:, dt, tt * 128:(tt + 1) * 128],
                        wv16[:, dt, g * 512:(g + 1) * 512],
                        start=(dt == 0), stop=(dt == DT - 1))
                for pp in range(4):
                    p = g * 4 + pp
                    nc.vector.tensor_copy(
                        vres[:, p, tt, 0:64], ps[:, pp * 128:pp * 128 + 64])
                    nc.vector.tensor_copy(
                        vres[:, p, tt, 65:129],
                        ps[:, pp * 128 + 64:(pp + 1) * 128])

            # prologue: K pair 0 + V pairs 0..3, then the c=0 sweep with the
            # remaining K pairs and V pairs 4..7 as pipeline fillers
            for th in kproj_fillers(0):
                th()
            for tt in range(KT):
                vproj_tt(0, tt)
            fillers = deque()
            for p in range(1, NPAIR):
                fillers.extend(kproj_fillers(p))
            fillers.extend(
                (lambda tt=tt: vproj_tt(1, tt)) for tt in range(KT))
            stream([(i, 0) for i in range(NPAIR)], fillers)
            assert not fillers

        # ---- c=1 sweep with the output projection as fillers ----
        wO = ctx.enter_context(tc.tile_pool(name="wO", bufs=1))
        wo16 = load_w("wo", wO)

        def dproj_fillers(qtile):
            thunks = []
            for dc in range(D // 512):
                st = {}
                def half_a(qtile=qtile, dc=dc, st=st):
                    st["ps"] = psM.tile([128, 512], F32, tag="mm", name="ops")
                    for p in range(4):
                        nc.tensor.matmul(
                            st["ps"][:], ctxT[:, p, qtile * 128:(qtile + 1) * 128],
                            wo16[:, p, dc * 512:(dc + 1) * 512],
                            start=(p == 0), stop=False)
                def half_b(qtile=qtile, dc=dc, st=st):
                    for p in range(4, NPAIR):
                        nc.tensor.matmul(
                            st["ps"][:], ctxT[:, p, qtile * 128:(qtile + 1) * 128],
                            wo16[:, p, dc * 512:(dc + 1) * 512],
                            start=False, stop=(p == NPAIR - 1))
                    ob = out_pool.tile([128, 512], F32, tag="ob", name="ob")
                    nc.vector.tensor_copy(ob[:], st["ps"][:])
                    nc.sync.dma_start(
                        out_d[qtile * 128:(qtile + 1) * 128,
                              dc * 512:(dc + 1) * 512], ob[:])
                thunks += [half_a, half_b]
            return thunks

        fillers = deque()
        for qtile in range(4):
            fillers.extend(dproj_fillers(qtile))
        stream([(i, 1) for i in range(NPAIR)], fillers)
        assert not fillers
        for qtile in range(4, QT):
            for th in dproj_fillers(qtile):
                th()

    return nc


# ---------------------------------------------------------------------------
# Host wrapper
# ---------------------------------------------------------------------------
from concourse.bass_utils import run_bass_kernel_spmd

B, S, D, H = 4, 2048, 1024, 16
SQ = S // 2
_NC = None
PROFILE = False
TRACE_DIR = None
LAST_EXEC_NS = None


def _get_nc():
    global _NC
    if _NC is None:
        _NC = build(S=S, SQ=SQ, D=D, H=H)
        legalize_waits(_NC)
    return _NC


def kernel(queries, keys, values, Wq, Wk, Wv, Wo):
    global LAST_EXEC_NS
    nc = _get_nc()
    bf16 = ml_dtypes.bfloat16
    q16 = np.asarray(queries, dtype=bf16)
    k16 = np.asarray(keys, dtype=bf16)
    v16 = np.asarray(values, dtype=bf16)
    w16 = {n: np.ascontiguousarray(np.asarray(w, dtype=bf16))
           for n, w in (("wq", Wq), ("wk", Wk), ("wv", Wv), ("wo", Wo))}
    in_maps = []
    for c in range(8):
        b, half = c // 2, c % 2
        in_maps.append({
            "q": np.ascontiguousarray(q16[b, half * SQ:(half + 1) * SQ, :]),
            "k": np.ascontiguousarray(k16[b]),
            "v": np.ascontiguousarray(v16[b]),
            **w16,
        })
    res = run_bass_kernel_spmd(nc, in_maps, list(range(8)), trace=PROFILE,
                               tmpdir=TRACE_DIR)
    LAST_EXEC_NS = res.exec_time_ns
    out = np.empty((B, S, D), np.float32)
    for c in range(8):
        out[c // 2, (c % 2) * SQ:(c % 2 + 1) * SQ, :] = res.results[c]["out"]
    return out
